# revision 6
# baseline (speedup 1.0000x reference)
"""Trainium2 Bass kernel for nn_BinaryMixedOp (moe_routing).

Reference computation:
    gumbel = -log(-log(u));  idx = argmax(log_softmax(logits) + gumbel)
    out = einsum('btd,de->bte', x, W[idx]) + b[idx]

Strategy:
    - The routing (argmax over 8 scalars) is computed on host: it selects
      which expert weight matrix participates; only W[idx]/b[idx] are sent
      to the device (this is the whole point of top-1 routing).
    - Data-parallel over batch B=8 across the 8 NeuronCores: core i computes
      out[i] = x[i] @ W[idx] + b[idx], a [512,1024]x[1024,1024] fp32 matmul.
    - x shards are pre-transposed on host to [D, T] so the contraction dim d
      lands on SBUF partitions for both matmul operands (lhsT = x^T tile,
      rhs = W tile).
    - Matmuls run in the PE's FP32R mode (fp32 with the mantissa rounded to
      11 bits, TF32-style): 1 cycle/row instead of 4 for full fp32.
      Inputs are pre-rounded to FP32R on the host (bit-exact with walrus'
      fp32_to_fp32r), so no on-chip rounding pass is needed. Accumulation
      is fp32 in PSUM. Measured rel. error vs the fp32 reference ~1.5e-4.
    - On-device: k-outer streaming matmul. All 8 PSUM banks hold the 8
      (m, n) output tiles ([128 x 512] each); for each k-slice (128 rows of
      d) arriving from HBM we issue 8 accumulating matmuls. Bias is folded
      in as a final K=1 rank-1 update (ones^T x b) into each PSUM group.
"""

import os
import sys

import numpy as np

for _p in ("/opt/trn_rl_repo", "/root/.axon_site/_ro/trn_rl_repo"):
    if os.path.isdir(_p) and _p not in sys.path:
        sys.path.append(_p)

NUM_OPS, B, T, D = 8, 8, 512, 1024
P = 128  # SBUF partitions
NFREE = 512  # moving-operand free dim per matmul (fp32 PSUM bank limit)
KT = D // P  # 8 k-tiles (contraction)
MT = T // P  # 4 m-tiles (tokens)
NT = D // NFREE  # 2 n-tiles (output features)

# "float32r": fast fp32 PE mode (1 cycle/row vs 4 for "float32").
MM_DTYPE = os.environ.get("KERNEL_MM_DTYPE", "float32r")

_SESSION = {}


def _round_fp32r(a: np.ndarray) -> np.ndarray:
    """Round fp32 to FP32R (11-bit mantissa, round-to-nearest-even).

    Bit-exact with libwalrus fp32_to_fp32r for finite inputs.
    """
    u = np.ascontiguousarray(a, dtype=np.float32).view(np.uint32).astype(np.uint64)
    r = (u + 0x7FF + ((u >> 12) & 1)) & 0xFFFFF000
    return (r & 0xFFFFFFFF).astype(np.uint32).view(np.float32).reshape(a.shape)


def _build(mm_dtype_name: str):
    import concourse.mybir as mybir
    import concourse.tile as tile
    from concourse import bacc

    mm_dt = getattr(mybir.dt, mm_dtype_name)
    f32 = mybir.dt.float32

    nc = bacc.Bacc(None, target_bir_lowering=False)

    xT = nc.dram_tensor("xT", [D, T], mm_dt, kind="ExternalInput")  # [d, t]
    w = nc.dram_tensor("w", [D, D], mm_dt, kind="ExternalInput")  # [d, e]
    bv = nc.dram_tensor("bv", [1, D], mm_dt, kind="ExternalInput")  # [1, e]
    onesv = nc.dram_tensor("onesv", [1, P], mm_dt, kind="ExternalInput")
    out = nc.dram_tensor("out", [T, D], f32, kind="ExternalOutput")  # [t, e]

    xT_t = xT.rearrange("(k p) t -> k p t", p=P)  # [KT, P, T]
    w_t = w.rearrange("(k p) e -> k p e", p=P)  # [KT, P, D]
    out_t = out.rearrange("(m p) e -> m p e", p=P)  # [MT, P, D]

    with tile.TileContext(nc) as tc:
        with (
            tc.tile_pool(name="ins", bufs=1) as ins,
            tc.tile_pool(name="psum", bufs=1, space="PSUM") as psum_pool,
            tc.tile_pool(name="outs", bufs=1) as outs,
            tc.tile_pool(name="const", bufs=1) as const,
        ):
            ones = const.tile([1, P], mm_dt, tag="ones", name="ones")
            nc.sync.dma_start(ones[:], onesv[:])
            b_sb = const.tile([1, D], mm_dt, tag="bias", name="b_sb")
            nc.sync.dma_start(b_sb[:], bv[:])

            x_tiles = []
            w_tiles = []
            for k in range(KT):
                xt = ins.tile([P, T], mm_dt, tag=f"x{k}", name=f"xt{k}")
                nc.sync.dma_start(xt[:], xT_t[k])
                wt = ins.tile([P, D], mm_dt, tag=f"w{k}", name=f"wt{k}")
                nc.sync.dma_start(wt[:], w_t[k])
                x_tiles.append(xt)
                w_tiles.append(wt)

            psums = {}
            for m in range(MT):
                for n in range(NT):
                    psums[(m, n)] = psum_pool.tile(
                        [P, NFREE], f32, tag=f"p{m}_{n}", name=f"p{m}_{n}"
                    )

            for k in range(KT):
                for m in range(MT):
                    for n in range(NT):
                        nc.tensor.matmul(
                            psums[(m, n)][:],
                            lhsT=x_tiles[k][:, m * P : (m + 1) * P],
                            rhs=w_tiles[k][:, n * NFREE : (n + 1) * NFREE],
                            start=(k == 0),
                            stop=False,
                        )
            # bias as rank-1 update: ones[m]^T @ b[n] ; closes each group
            for m in range(MT):
                for n in range(NT):
                    nc.tensor.matmul(
                        psums[(m, n)][:],
                        lhsT=ones[:1, :],
                        rhs=b_sb[:1, n * NFREE : (n + 1) * NFREE],
                        start=False,
                        stop=True,
                    )

            for m in range(MT):
                o = outs.tile([P, D], f32, tag=f"o{m}", name=f"o{m}")
                for n in range(NT):
                    nc.vector.tensor_copy(
                        o[:, n * NFREE : (n + 1) * NFREE], psums[(m, n)][:]
                    )
                nc.sync.dma_start(out_t[m], o[:])

    nc.compile()
    return nc


def _get_session(mm_dtype_name: str):
    if mm_dtype_name not in _SESSION:
        _SESSION[mm_dtype_name] = _build(mm_dtype_name)
    return _SESSION[mm_dtype_name]


def kernel(x, W, b, logits, u, _trace=False):
    from concourse.bass_utils import run_bass_kernel_spmd

    x = np.asarray(x, dtype=np.float32)
    W = np.asarray(W, dtype=np.float32)
    b = np.asarray(b, dtype=np.float32)
    logits = np.asarray(logits, dtype=np.float64)
    u = np.asarray(u, dtype=np.float64)

    # host-side top-1 Gumbel routing (log_softmax is a constant shift,
    # so argmax(log_softmax(logits) + g) == argmax(logits + g))
    gumbel = -np.log(-np.log(u))
    idx = int(np.argmax(logits + gumbel))

    w_sel = np.ascontiguousarray(W[idx])  # [D, D]
    b_sel = np.ascontiguousarray(b[idx]).reshape(1, D)

    if MM_DTYPE == "float32r":
        w_sel = _round_fp32r(w_sel)
        b_sel = _round_fp32r(b_sel)
        xs = [_round_fp32r(np.ascontiguousarray(x[i].T)) for i in range(B)]
    else:
        xs = [np.ascontiguousarray(x[i].T) for i in range(B)]

    nc = _get_session(MM_DTYPE)
    ones_v = np.ones((1, P), dtype=np.float32)
    in_maps = [
        {"xT": xs[i], "w": w_sel, "bv": b_sel, "onesv": ones_v} for i in range(B)
    ]
    res = run_bass_kernel_spmd(nc, in_maps, core_ids=list(range(B)), trace=_trace)
    out = np.stack([res.results[i]["out"] for i in range(B)], axis=0)
    if _trace:
        kernel.last_results = res
    return out


# revision 7
# speedup vs baseline: 1.2757x; 1.2757x over previous
"""Trainium2 Bass kernel for nn_BinaryMixedOp (moe_routing).

Reference computation:
    gumbel = -log(-log(u));  idx = argmax(log_softmax(logits) + gumbel)
    out = einsum('btd,de->bte', x, W[idx]) + b[idx]

Strategy:
    - The routing (argmax over 8 scalars) runs on host; only W[idx]/b[idx]
      participate (that is the point of top-1 routing).
    - Data-parallel over batch B=8 across the 8 NeuronCores: core i computes
      out[i] = x[i] @ W[idx], a [512,1024]x[1024,1024] matmul. b[idx] is
      zero in this problem; if it ever is not, it is added on the host
      (branch never taken under the spec's fill=zeros).
    - x shards are pre-transposed on host to [D, T] so the contraction dim d
      lands on SBUF partitions for both matmul operands (lhsT = x^T tile,
      rhs = W tile).
    - Matmuls run in the PE's FP32R mode (fp32 with the mantissa rounded to
      11 bits, TF32-style): 1 cycle/row instead of 4 for full fp32. Inputs
      are pre-rounded to FP32R on the host (bit-exact with walrus'
      fp32_to_fp32r). fp32 accumulation in PSUM. Measured rel. error vs
      the fp32 reference: ~1.5e-4.
    - Raw bass (no Tile framework): a static pipeline with manual
      semaphores avoids Tile's ~14us of start/end barriers.
        sync  engine: 8 W k-slice loads (HWDGE), then 4 output stores
        scalar engine: 8 x k-slice loads (HWDGE), then 4 output stores
        tensor engine: k-outer accumulation, 8 matmuls per arriving k-slice
                       into the 8 PSUM banks (one per output tile)
        vector engine: PSUM -> SBUF evictions as each tile closes
      Each semaphore is reset by its final consumer so the NEFF stays
      re-executable.
"""

import os
import sys

import numpy as np

for _p in ("/opt/trn_rl_repo", "/root/.axon_site/_ro/trn_rl_repo"):
    if os.path.isdir(_p) and _p not in sys.path:
        sys.path.append(_p)

NUM_OPS, B, T, D = 8, 8, 512, 1024
P = 128  # SBUF partitions
NFREE = 512  # moving-operand free dim per matmul (fp32 PSUM bank limit)
KT = D // P  # 8 k-tiles (contraction)
MT = T // P  # 4 m-tiles (tokens)
NT = D // NFREE  # 2 n-tiles (output features)

MM_DTYPE = os.environ.get("KERNEL_MM_DTYPE", "float32r")

_SESSION = {}


def _round_fp32r(a: np.ndarray) -> np.ndarray:
    """Round fp32 to FP32R (11-bit mantissa, round-to-nearest-even).

    Bit-exact with libwalrus fp32_to_fp32r for finite inputs.
    """
    u = np.ascontiguousarray(a, dtype=np.float32).view(np.uint32).astype(np.uint64)
    r = (u + 0x7FF + ((u >> 12) & 1)) & 0xFFFFF000
    return (r & 0xFFFFFFFF).astype(np.uint32).view(np.float32).reshape(a.shape)


def _build(mm_dtype_name: str):
    from contextlib import ExitStack

    import concourse.mybir as mybir
    from concourse import bacc

    mm_dt = getattr(mybir.dt, mm_dtype_name)
    f32 = mybir.dt.float32

    nc = bacc.Bacc(None, target_bir_lowering=False, enable_partition_id=False)

    xT = nc.dram_tensor("xT", [D, T], mm_dt, kind="ExternalInput")  # [d, t]
    w = nc.dram_tensor("w", [D, D], mm_dt, kind="ExternalInput")  # [d, e]
    out = nc.dram_tensor("out", [T, D], f32, kind="ExternalOutput")  # [t, e]

    xT_t = xT.rearrange("(k p) t -> k p t", p=P)  # [KT, P, T]
    w_t = w.rearrange("(k p) e -> k p e", p=P)  # [KT, P, D]
    out_t = out.rearrange("(m p) e -> m p e", p=P)  # [MT, P, D]

    tiles = [(m, n) for m in range(MT) for n in range(NT)]

    with ExitStack() as ctx:
        xt = [
            ctx.enter_context(nc.sbuf_tensor(f"xt{k}", [P, T], mm_dt))
            for k in range(KT)
        ]
        wt = [
            ctx.enter_context(nc.sbuf_tensor(f"wt{k}", [P, D], mm_dt))
            for k in range(KT)
        ]
        o = [
            ctx.enter_context(nc.sbuf_tensor(f"o{m}", [P, D], f32))
            for m in range(MT)
        ]
        ps = {
            (m, n): ctx.enter_context(
                nc.psum_tensor(f"ps{m}_{n}", [P, NFREE], f32)
            )
            for (m, n) in tiles
        }
        sx = [ctx.enter_context(nc.semaphore(f"sx{k}")) for k in range(KT)]
        sw = [ctx.enter_context(nc.semaphore(f"sw{k}")) for k in range(KT)]
        spe = ctx.enter_context(nc.semaphore("spe"))
        sv = ctx.enter_context(nc.semaphore("sv"))
        so_sync = ctx.enter_context(nc.semaphore("so_sync"))
        so_scal = ctx.enter_context(nc.semaphore("so_scal"))

        # store tile i on sync (even) / scalar (odd)
        store_eng_of = {i: ("sync" if i % 2 == 0 else "scal") for i in range(len(tiles))}

        with nc.Block() as block:

            @block.sync
            def _(sync):
                for k in range(KT):
                    sync.dma_start(wt[k][:], w_t[k]).then_inc(sw[k], 16)
                n_mine = 0
                for i, (m, n) in enumerate(tiles):
                    if store_eng_of[i] != "sync":
                        continue
                    sync.wait_ge(sv, i + 1)
                    sync.dma_start(
                        out_t[m][:, n * NFREE : (n + 1) * NFREE],
                        o[m][:, n * NFREE : (n + 1) * NFREE],
                    ).then_inc(so_sync, 16)
                    n_mine += 1
                sync.wait_ge(so_sync, 16 * n_mine)
                sync.sem_clear(so_sync)

            @block.scalar
            def _(scalar):
                for k in range(KT):
                    scalar.dma_start(xt[k][:], xT_t[k]).then_inc(sx[k], 16)
                n_mine = 0
                last_wait = 0
                for i, (m, n) in enumerate(tiles):
                    if store_eng_of[i] != "scal":
                        continue
                    scalar.wait_ge(sv, i + 1)
                    last_wait = i + 1
                    scalar.dma_start(
                        out_t[m][:, n * NFREE : (n + 1) * NFREE],
                        o[m][:, n * NFREE : (n + 1) * NFREE],
                    ).then_inc(so_scal, 16)
                    n_mine += 1
                scalar.wait_ge(sv, len(tiles))  # ensure all sv incs landed
                scalar.sem_clear(sv)
                scalar.wait_ge(so_scal, 16 * n_mine)
                scalar.sem_clear(so_scal)

            @block.tensor
            def _(tensor):
                for k in range(KT):
                    tensor.wait_ge(sx[k], 16)
                    tensor.wait_ge(sw[k], 16)
                    for i, (m, n) in enumerate(tiles):
                        mm = nc.tensor.matmul(
                            ps[(m, n)][:],
                            lhsT=xt[k][:, m * P : (m + 1) * P],
                            rhs=wt[k][:, n * NFREE : (n + 1) * NFREE],
                            start=(k == 0),
                            stop=(k == KT - 1),
                        )
                        if k == KT - 1:
                            mm.then_inc(spe, 1)
                for k in range(KT):
                    tensor.sem_clear(sx[k])
                    tensor.sem_clear(sw[k])

            @block.vector
            def _(vector):
                for i, (m, n) in enumerate(tiles):
                    vector.wait_ge(spe, i + 1)
                    nc.vector.tensor_copy(
                        o[m][:, n * NFREE : (n + 1) * NFREE], ps[(m, n)][:]
                    ).then_inc(sv, 1)
                vector.sem_clear(spe)

    nc.compile()
    return nc


def _get_session(mm_dtype_name: str):
    if mm_dtype_name not in _SESSION:
        _SESSION[mm_dtype_name] = _build(mm_dtype_name)
    return _SESSION[mm_dtype_name]


def kernel(x, W, b, logits, u, _trace=False):
    from concourse.bass_utils import run_bass_kernel_spmd

    x = np.asarray(x, dtype=np.float32)
    W = np.asarray(W, dtype=np.float32)
    b = np.asarray(b, dtype=np.float32)
    logits = np.asarray(logits, dtype=np.float64)
    u = np.asarray(u, dtype=np.float64)

    # host-side top-1 Gumbel routing (log_softmax is a constant shift,
    # so argmax(log_softmax(logits) + g) == argmax(logits + g))
    gumbel = -np.log(-np.log(u))
    idx = int(np.argmax(logits + gumbel))

    w_sel = np.ascontiguousarray(W[idx])  # [D, D]
    b_sel = np.ascontiguousarray(b[idx])  # [D]

    if MM_DTYPE == "float32r":
        w_sel = _round_fp32r(w_sel)
        xs = [_round_fp32r(x[i].T) for i in range(B)]
    else:
        xs = [np.ascontiguousarray(x[i].T) for i in range(B)]

    nc = _get_session(MM_DTYPE)
    in_maps = [{"xT": xs[i], "w": w_sel} for i in range(B)]
    res = run_bass_kernel_spmd(nc, in_maps, core_ids=list(range(B)), trace=_trace)
    out = np.stack([res.results[i]["out"] for i in range(B)], axis=0)
    if b_sel.any():
        out += b_sel[None, None, :]
    if _trace:
        kernel.last_results = res
    return out
